# revision 16
# baseline (speedup 1.0000x reference)
"""Trainium2 Bass kernel for nn_Example1 (dense_transformer relation attention).

Reference math (b=32, n=1024, VOCAB=2048, D=3072):
    enc[b, j] = onehot(token[b, j], VOCAB) ++ onehot(j, n)          # 2 ones per row
    A = softmax_j(enc R enc^T + causal)
    logits = (A @ enc)[:, -1, :]

Only the LAST query row survives to the output, and enc is 2-hot, so the
whole computation collapses to (per sequence, t = token ids, tl = t[1023]):
    q       = R[tl, :] + R[3071, :]                       # row gather
    s[j]    = q[t_j] + q[2048 + j]                        # element gather
    A[j]    = softmax(s)[j]                               # last row unmasked
    out[2048 + j] = A[j]
    out[v]  = sum_{j: t_j == v} A[j]   for v < 2048        # weighted histogram

Device mapping (8 NeuronCores, data-parallel over batch, 4 sequences/core):
    - R row gather:   GPSIMD indirect DMA (data-dependent row offsets)
    - element gather: GPSIMD ap_gather from a per-batch SBUF table
    - softmax:        ScalarE exp with fused row-sum + DVE reciprocal
    - histogram:      one-hot decomposition 2048 = 64*32; TensorE matmuls
                      hist[a, c] = sum_j [t_j>>5 == a] * ([t_j&31 == c] * A_j)

kernel(**inputs) takes FULL inputs (token_ids [32, 1024] int, R [3072, 3072]
f32) and returns the FULL [32, 3072] f32 output. Host side only reshapes /
shards; all data-dependent compute runs on device.
"""

from contextlib import ExitStack

import numpy as np

import concourse.bacc as bacc
import concourse.bass as bass
import concourse.mybir as mybir
import concourse.tile as tile
from concourse import library_config
from concourse.bass_utils import run_bass_kernel_spmd
from concourse.masks import make_identity

VOCAB = 2048
CTX = 1024
D = VOCAB + CTX  # 3072
NCORES = 8
BPC = 4  # batches (sequences) per core

F32 = mybir.dt.float32
I32 = mybir.dt.int32
I16 = mybir.dt.int16
OP = mybir.AluOpType


def _emit(nc, tokw, tokc, tl4, tl128, R, out):
    """Emit the per-core kernel. tokw [128,32] i16 wrapped token idxs for
    ap_gather; tokc [128,32] i32 tokens with j on partitions (col = 8b+k,
    j = 128k + jj); tl4 [4,1], tl128 [128,1] i32 last-token ids;
    R [3072,3072] f32; out [4,3072] f32."""
    with tile.TileContext(nc) as tc, ExitStack() as ctx:
        pool = ctx.enter_context(tc.tile_pool(name="main", bufs=1))
        wpool = ctx.enter_context(tc.tile_pool(name="work", bufs=3))
        ppool = ctx.enter_context(tc.tile_pool(name="psum", bufs=2, space="PSUM"))
        hpool = ctx.enter_context(tc.tile_pool(name="hist", bufs=2, space="PSUM"))

        nc.gpsimd.load_library(library_config.ap_gather)

        # ---- constants ----
        ident = pool.tile([128, 128], F32, name="ident")
        make_identity(nc, ident[:])
        i_a = pool.tile([128, 64], I32, name="i_a")
        nc.gpsimd.iota(i_a[:], pattern=[[32, 64]], base=0, channel_multiplier=0)
        i_af = pool.tile([128, 64], F32, name="i_af")
        nc.vector.tensor_copy(i_af[:], i_a[:])
        i_c = pool.tile([128, 32], I32, name="i_c")
        nc.gpsimd.iota(i_c[:], pattern=[[1, 32]], base=0, channel_multiplier=0)
        i_cf = pool.tile([128, 32], F32, name="i_cf")
        nc.vector.tensor_copy(i_cf[:], i_c[:])

        # ---- token tiles ----
        tokw_s = pool.tile([128, 32], I16, name="tokw_s")
        nc.sync.dma_start(tokw_s[:], tokw)
        tokc_s = pool.tile([128, 32], I32, name="tokc_s")
        nc.sync.dma_start(tokc_s[:], tokc)

        # last-token row indices (host-marshalled layouts of t[:, 1023]):
        # tl4[b] = t[b, 1023];  tl128[16*(b+4h) + r] = t[b, 1023]
        # (ap_gather core c = b + 4h serves batch b, j-half h)
        ri = pool.tile([BPC, 1], I32, name="ri")
        nc.sync.dma_start(ri[:], tl4)
        idx128 = pool.tile([128, 1], I32, name="idx128")
        nc.sync.dma_start(idx128[:], tl128)

        # ---- gather table (vocab part of q, replicated 16x per (b, h)):
        # tbl[16*(b+4h) + r, v] = R[tl_b, v] + R[3071, v]
        tbl = pool.tile([128, VOCAB], F32, name="tbl")
        nc.gpsimd.indirect_dma_start(
            out=tbl[:], out_offset=None, in_=R,
            in_offset=bass.IndirectOffsetOnAxis(ap=idx128[:, 0:1], axis=0),
        )
        r71b = pool.tile([128, VOCAB], F32, name="r71b")
        r71b_src = bass.AP(tensor=R.tensor, offset=3071 * D, ap=[[0, 128], [1, VOCAB]])
        nc.sync.dma_start(r71b[:], r71b_src)
        nc.vector.tensor_tensor(out=tbl[:], in0=tbl[:], in1=r71b[:], op=OP.add)

        # ---- positional part of q: q4p[b, j] = R[tl_b, 2048+j] + R[3071, 2048+j]
        rtlp = pool.tile([BPC, CTX], F32, name="rtlp")
        nc.gpsimd.indirect_dma_start(
            out=rtlp[:], out_offset=None, in_=R,
            in_offset=bass.IndirectOffsetOnAxis(ap=ri[:, 0:1], axis=0),
            element_offset=VOCAB,
        )
        r71p = pool.tile([BPC, CTX], F32, name="r71p")
        r71p_src = bass.AP(tensor=R.tensor, offset=3071 * D + VOCAB,
                           ap=[[0, BPC], [1, CTX]])
        nc.sync.dma_start(r71p[:], r71p_src)
        q4p = pool.tile([BPC, CTX], F32, name="q4p")
        nc.vector.tensor_tensor(out=q4p[:], in0=rtlp[:], in1=r71p[:], op=OP.add)

        # ---- element gather: core c=b+4h gathers q_b[t] for its 512 j ----
        gq = pool.tile([128, 512], F32, name="gq")
        nc.gpsimd.ap_gather(
            out_ap=gq[:].rearrange("c (i d) -> c i d", d=1),
            in_ap=tbl[:].rearrange("c (n d) -> c n d", d=1),
            idxs_ap=tokw_s[:],
            channels=128, num_elems=VOCAB, d=1, num_idxs=512,
        )

        # ---- collect the 8 useful gather rows into [4, 1024] (DMA: engine ops
        # cannot read strided partitions at non-32-aligned bases) ----
        gqc = pool.tile([BPC, CTX], F32, name="gqc")
        nc.sync.dma_start(gqc[:, 0:512], gq[0:64:16, :])
        nc.sync.dma_start(gqc[:, 512:CTX], gq[64:128:16, :])

        # ---- scores [4, 1024]: s = gq + q[2048 + j] ----
        s4 = pool.tile([BPC, CTX], F32, name="s4")
        nc.vector.tensor_tensor(out=s4[:], in0=gqc[:], in1=q4p[:], op=OP.add)

        # ---- softmax (scores are tiny: skip max-subtraction) ----
        e4 = pool.tile([BPC, CTX], F32, name="e4")
        ssum = pool.tile([BPC, 1], F32, name="ssum")
        nc.scalar.activation(e4[:], s4[:], mybir.ActivationFunctionType.Exp,
                             accum_out=ssum[:])
        srec = pool.tile([BPC, 1], F32, name="srec")
        nc.vector.reciprocal(srec[:], ssum[:])
        a4 = pool.tile([BPC, CTX], F32, name="a4")
        nc.vector.tensor_scalar(out=a4[:], in0=e4[:], scalar1=srec[:, 0:1],
                                scalar2=None, op0=OP.mult)
        # positional half of the output
        nc.sync.dma_start(out[:, VOCAB:D], a4[:])

        # ---- transpose A to j-on-partitions: ac[jj, 8b+k] = A[b, 128k+jj] ----
        ac = pool.tile([128, 32], F32, name="ac")
        for k in range(8):
            tp = ppool.tile([128, BPC], F32, name="tp")
            nc.tensor.transpose(out=tp[:], in_=a4[:, 128 * k:128 * (k + 1)],
                                identity=ident[0:BPC, 0:BPC])
            nc.scalar.copy(out=ac[:, k:32:8], in_=tp[:])

        # ---- histogram one-hot pieces: a = t >> 5 (as 32a), c = t & 31 ----
        ci = pool.tile([128, 32], I32, name="ci")
        nc.vector.tensor_scalar(out=ci[:], in0=tokc_s[:], scalar1=31,
                                scalar2=None, op0=OP.bitwise_and)
        cf = pool.tile([128, 32], F32, name="cf")
        nc.vector.tensor_copy(cf[:], ci[:])
        df = pool.tile([128, 32], F32, name="df")  # 32*a = t - c, exact
        nc.vector.tensor_tensor(out=df[:], in0=tokc_s[:], in1=ci[:], op=OP.subtract)

        # ---- hist[a, c] = sum_j oneA[j, a] * (oneC[j, c] * A_j) ----
        for b in range(BPC):
            hp = hpool.tile([64, 32], F32, name="hp")
            for k in range(8):
                col = 8 * b + k
                one_a = wpool.tile([128, 64], F32, name="one_a")
                nc.vector.tensor_scalar(out=one_a[:], in0=i_af[:],
                                        scalar1=df[:, col:col + 1],
                                        scalar2=None, op0=OP.is_equal)
                w = wpool.tile([128, 32], F32, name="w")
                nc.vector.tensor_scalar(out=w[:], in0=i_cf[:],
                                        scalar1=cf[:, col:col + 1],
                                        scalar2=ac[:, col:col + 1],
                                        op0=OP.is_equal, op1=OP.mult)
                nc.tensor.matmul(out=hp[:], lhsT=one_a[:], rhs=w[:],
                                 start=(k == 0), stop=(k == 7))
            hs = wpool.tile([64, 32], F32, name="hs")
            nc.scalar.copy(out=hs[:], in_=hp[:])
            dst = out[b:b + 1, 0:VOCAB].rearrange("one (a c) -> (one a) c", c=32)
            nc.sync.dma_start(dst, hs[:])


def build_nc():
    nc = bacc.Bacc("TRN2", target_bir_lowering=False, debug=False)
    tokw = nc.dram_tensor("tokw", [128, 32], I16, kind="ExternalInput")
    tokc = nc.dram_tensor("tokc", [128, 32], I32, kind="ExternalInput")
    tl4 = nc.dram_tensor("tl4", [BPC, 1], I32, kind="ExternalInput")
    tl128 = nc.dram_tensor("tl128", [128, 1], I32, kind="ExternalInput")
    R = nc.dram_tensor("R", [D, D], F32, kind="ExternalInput")
    out = nc.dram_tensor("out", [BPC, D], F32, kind="ExternalOutput")
    _emit(nc, tokw.ap()[:, :], tokc.ap()[:, :], tl4.ap()[:, :],
          tl128.ap()[:, :], R.ap()[:, :], out.ap()[:, :])
    nc.compile()
    return nc


_NC_CACHE = None


def _get_nc():
    global _NC_CACHE
    if _NC_CACHE is None:
        _NC_CACHE = build_nc()
    return _NC_CACHE


def _make_in_maps(token_ids, R):
    token_ids = np.asarray(token_ids).astype(np.int32)
    R = np.ascontiguousarray(np.asarray(R, dtype=np.float32))
    assert token_ids.shape == (NCORES * BPC, CTX), token_ids.shape
    assert R.shape == (D, D), R.shape
    in_maps = []
    for c in range(NCORES):
        t = token_ids[c * BPC:(c + 1) * BPC]  # [4, 1024]
        # tokw[16*(b+4h)+r, s] = t[b, 512h+16s+r]  (ap_gather wrapped layout)
        tw = t.reshape(BPC, 2, 32, 16).transpose(1, 0, 3, 2).reshape(128, 32)
        # tokc[jj, 8b+k] = t[b, 128k+jj]
        tcc = t.reshape(BPC, 8, 128).transpose(2, 0, 1).reshape(128, 32)
        tl = t[:, -1].astype(np.int32)  # [4]
        tl128 = np.repeat(np.tile(tl, 2), 16).reshape(128, 1)
        in_maps.append({
            "tokw": np.ascontiguousarray(tw.astype(np.int16)),
            "tokc": np.ascontiguousarray(tcc.astype(np.int32)),
            "tl4": np.ascontiguousarray(tl.reshape(BPC, 1)),
            "tl128": np.ascontiguousarray(tl128),
            "R": R,
        })
    return in_maps


def _run(token_ids, R, trace=False):
    nc = _get_nc()
    in_maps = _make_in_maps(token_ids, R)
    res = run_bass_kernel_spmd(nc, in_maps, list(range(NCORES)), trace=trace)
    full = np.concatenate([res.results[c]["out"] for c in range(NCORES)], axis=0)
    return full, res


def kernel(**inputs):
    token_ids = inputs["token_ids"]
    R = inputs["R"]
    full, _ = _run(token_ids, R, trace=False)
    return full


def kernel_profiled(**inputs):
    """Like kernel() but also returns the profiled HW exec time in ns."""
    full, res = _run(inputs["token_ids"], inputs["R"], trace=True)
    return full, res.exec_time_ns
